# revision 29
# baseline (speedup 1.0000x reference)
"""Multi-head attention with RoPE (B=2, S=2048, H=16 heads, D=64) on 8 TRN2
NeuronCores, tensor-parallel over heads (2 heads/core).

Per core c (heads 2c, 2c+1), all matmul paths in fp16 (fp32 PSUM accum):
  qT/kT = W_slice @ x^T and v (natural layout) from one shared fp16 x^T input
  RoPE on qT/kT: out = t*cos + swap32(t)*sin_signed (swap via SBUF->SBUF DMA)
  scores^T[k, q] = kT.T @ qT per head, exp via ScalarE -> P^T (fp16)
  ctxT[d, q] (+ row of column-sums l) = [v | 1].T @ P^T
  normalize ctxT by 1/l (reciprocal in [128,8] layout, broadcast via DRAM),
  project with Wo slice -> partial output [4096, 1024]; host sums 8 partials.
"""
import numpy as np
import ml_dtypes

import concourse.bass as bass
import concourse.mybir as mybir
import concourse.tile as tile
from concourse import bacc
from concourse.bass_utils import run_bass_kernel_spmd

F32 = mybir.dt.float32
F16 = mybir.dt.float16

B, S, HID = 2, 2048, 1024
NH, HD = 16, 64
T = B * S                  # 4096 tokens
NCORES = 8
HPC = NH // NCORES         # 2 heads per core
DPC = HPC * HD             # 128 context dims per core
ROPE_BASE = 10000.0

_CACHE = {}
DEBUG_TAPS = False


def _build_program():
    nc = bacc.Bacc("TRN2", target_bir_lowering=False, debug=False)

    xT_d = nc.dram_tensor("xT16", [HID, T], F16, kind="ExternalInput")
    wq_d = nc.dram_tensor("wq", [128, HID], F16, kind="ExternalInput")
    wk_d = nc.dram_tensor("wk", [128, HID], F16, kind="ExternalInput")
    wv_d = nc.dram_tensor("wv", [128, HID], F16, kind="ExternalInput")
    wo_d = nc.dram_tensor("wo", [DPC, HID], F16, kind="ExternalInput")
    cos_d = nc.dram_tensor("cosf", [128, S], F16, kind="ExternalInput")
    sin_d = nc.dram_tensor("sins", [128, S], F16, kind="ExternalInput")
    out_d = nc.dram_tensor("out", [T, HID], F32, kind="ExternalOutput")
    lscr_d = nc.dram_tensor("lscr", [8, 1024], F32)   # l rows bounce
    wscr_d = nc.dram_tensor("wscr", [128, 512], F32)  # warmup sink
    rscr_d = nc.dram_tensor("rscr", [8, 1024], F32)   # 1/l rows bounce

    dbg = None
    if DEBUG_TAPS:
        dbg = {
            "qT": nc.dram_tensor("dbg_qT", [128, T], F32, kind="ExternalOutput"),
            "kT": nc.dram_tensor("dbg_kT", [128, T], F32, kind="ExternalOutput"),
            "v0": nc.dram_tensor("dbg_v0", [128, 65], F32, kind="ExternalOutput"),
            "ctx0": nc.dram_tensor("dbg_ctx0", [128, S], F32, kind="ExternalOutput"),
            "lscr": nc.dram_tensor("dbg_lscr", [8, 1024], F32, kind="ExternalOutput"),
            "sp": nc.dram_tensor("dbg_sp", [128, 1024], F32, kind="ExternalOutput"),
            "p": nc.dram_tensor("dbg_p", [128, 1024], F32, kind="ExternalOutput"),
            "ctxps": nc.dram_tensor("dbg_ctxps", [65, 1024], F32, kind="ExternalOutput"),
        }

    with tile.TileContext(nc) as tc:
        _emit(nc, tc, xT_d, wq_d, wk_d, wv_d, wo_d, cos_d, sin_d, out_d,
              lscr_d, rscr_d, wscr_d, dbg=dbg)
    nc.compile()
    return nc


def _emit(nc, tc, xT_d, wq_d, wk_d, wv_d, wo_d, cos_d, sin_d, out_d,
          lscr_d, rscr_d, wscr_d, dbg=None):
    import contextlib
    ctx = contextlib.ExitStack()
    with ctx:
        singles = ctx.enter_context(tc.tile_pool(name="singles", bufs=1))
        xpool = ctx.enter_context(tc.tile_pool(name="xpool", bufs=12))
        ppool = ctx.enter_context(tc.tile_pool(name="ppool", bufs=8))
        rotp = ctx.enter_context(tc.tile_pool(name="rotp", bufs=2))
        lpool = ctx.enter_context(tc.tile_pool(name="lpool", bufs=4))
        bpool = ctx.enter_context(tc.tile_pool(name="bpool", bufs=4))
        opool = ctx.enter_context(tc.tile_pool(name="opool", bufs=12))

        # ---- persistent SBUF ----
        wq_sb = singles.tile([128, 8, DPC], F16)
        wk_sb = singles.tile([128, 8, DPC], F16)
        wv_sb = singles.tile([128, 8, DPC], F16)
        wo_sb = singles.tile([128, HID], F16)
        cos_sb = singles.tile([128, S], F16)
        sin_sb = singles.tile([128, S], F16)
        qT_sb = singles.tile([128, T], F16)
        kT_sb = singles.tile([128, T], F16)
        v_all = singles.tile([128, 64, 65], F16)  # v natural, slot = h*32+b*16+kb
        ctx0_sb = singles.tile([128, S], F16)     # normalized ctx^T for b=0
        ctx1_sb = singles.tile([128, S], F16)

        nc.sync.dma_start(out=wq_sb[:].rearrange("p a b -> p (a b)"), in_=wq_d[:])
        nc.scalar.dma_start(out=wk_sb[:].rearrange("p a b -> p (a b)"), in_=wk_d[:])
        nc.gpsimd.dma_start(out=wv_sb[:].rearrange("p a b -> p (a b)"), in_=wv_d[:])
        nc.gpsimd.dma_start(out=wo_sb[:], in_=wo_d[:])
        nc.gpsimd.dma_start(out=cos_sb[:], in_=cos_d[:])
        nc.gpsimd.dma_start(out=sin_sb[:], in_=sin_d[:])
        nc.vector.memset(v_all[:, :, 64:65], 1.0)
        tblw = singles.tile([1, 8], F32)
        nc.vector.memset(tblw[:], 0.0)
        nc.scalar.activation(out=tblw[:], in_=tblw[:],
                             func=mybir.ActivationFunctionType.Exp)

        # ---- phase 1: q/k (T-layout) + v (natural) from shared x tiles,
        # with RoPE fused per batch-half to keep the PE dense ----
        def rope_one(t_sb, b, part=None):
            # part=None: all; part=k (0..3): quarter k of the DVE mul/add work
            bsl = slice(b * S, (b + 1) * S)
            if part in (None, 0):
                rot = rotp.tile([128, S], F16, tag=f"rot{b}", name="rot")
                rotp_cache[b] = rot
                nc.sync.dma_start(out=rot[0:32, :], in_=t_sb[32:64, bsl])
                nc.sync.dma_start(out=rot[32:64, :], in_=t_sb[0:32, bsl])
                nc.sync.dma_start(out=rot[64:96, :], in_=t_sb[96:128, bsl])
                nc.sync.dma_start(out=rot[96:128, :], in_=t_sb[64:96, bsl])
                nc.gpsimd.tensor_mul(rot[:], rot[:], sin_sb[:])
            rot = rotp_cache[b]
            if part is None:
                cols = [slice(0, S)]
            else:
                half = part % 2
                cols = [slice(half * (S // 2), (half + 1) * (S // 2))]
            for cs in cols:
                tsl2 = slice(b * S + cs.start, b * S + cs.stop)
                nc.vector.tensor_mul(t_sb[:, tsl2], t_sb[:, tsl2], cos_sb[:, cs])
                nc.vector.tensor_add(t_sb[:, tsl2], t_sb[:, tsl2], rot[:, cs])

        rotp_cache = {}

        def rope_half(b):
            rope_one(qT_sb, b)
            rope_one(kT_sb, b)

        with tc.tile_pool(name="qkps", bufs=2, space="PSUM") as qkps, \
             tc.tile_pool(name="vps", bufs=1, space="PSUM") as vps:
            with nc.named_scope("qkv"):
                for tcn in range(8):  # token chunks of 512
                    tsl = slice(tcn * 512, (tcn + 1) * 512)
                    psq = qkps.tile([128, 512], F32)
                    psk = qkps.tile([128, 512], F32)
                    pvs = [vps.tile([128, DPC], F32, tag=f"pv{i}", name=f"pv{i}")
                           for i in range(4)]
                    for kc in range(8):
                        xt = xpool.tile([128, 512], F16)
                        dma_eng = nc.sync if kc % 2 == 0 else nc.scalar
                        dma_eng.dma_start(
                            out=xt[:],
                            in_=xT_d[kc * 128:(kc + 1) * 128, tsl])
                        st, sp = kc == 0, kc == 7
                        nc.tensor.matmul(psq[:], wq_sb[:, kc, :], xt[:], start=st, stop=sp)
                        nc.tensor.matmul(psk[:], wk_sb[:, kc, :], xt[:], start=st, stop=sp)
                        for sub in range(4):
                            nc.tensor.matmul(
                                pvs[sub][:],
                                xt[:, sub * 128:(sub + 1) * 128],
                                wv_sb[:, kc, :],
                                start=st, stop=sp)
                    nc.vector.tensor_copy(qT_sb[:, tsl], psq[:])
                    nc.vector.tensor_copy(kT_sb[:, tsl], psk[:])
                    for sub in range(4):
                        blk = tcn * 4 + sub
                        dst0 = v_all[:, blk, 0:64]
                        dst = bass.AP(tensor=dst0.tensor, offset=dst0.offset,
                                      ap=[list(dst0.ap[0]), [32 * 65, 2], [1, 64]])
                        nc.vector.tensor_copy(dst, pvs[sub][:, 0:128])
                    if tcn == 3:
                        rope_one(qT_sb, 0, part=0)
                    if tcn == 4:
                        rope_one(qT_sb, 0, part=1)
                    if tcn == 5:
                        rope_one(kT_sb, 0, part=0)
                    if tcn == 6:
                        rope_one(kT_sb, 0, part=1)
            with nc.named_scope("rope1"):
                rope_half(1)

        # ---- phases 3+4: attention + output projection, per batch ----
        projq = []

        def emit_proj_unit(pool, copy_eng=None, dma_eng=None, tag="sp"):
            bb, qb, oc = projq.pop(0)
            src = ctx0_sb if bb == 0 else ctx1_sb
            qsl = slice(qb * 128, (qb + 1) * 128)
            osl = slice(oc * 512, (oc + 1) * 512)
            ops = pool.tile([128, 512], F32, tag=tag, name="ops")
            nc.tensor.matmul(ops[:], src[:, qsl], wo_sb[:, osl],
                             start=True, stop=True)
            ot = opool.tile([128, 512], F32, tag="ot", name="ot")
            if copy_eng is nc.scalar:
                nc.scalar.activation(out=ot[:], in_=ops[:],
                                     func=mybir.ActivationFunctionType.Copy)
            else:
                nc.vector.tensor_copy(ot[:], ops[:])
            (dma_eng or nc.sync).dma_start(
                out=out_d[bb * S + qb * 128:bb * S + (qb + 1) * 128, osl],
                in_=ot[:])

        with tc.tile_pool(name="aps", bufs=2, space="PSUM") as aps, \
             tc.tile_pool(name="cps", bufs=1, space="PSUM") as cps, \
             tc.tile_pool(name="pps", bufs=2, space="PSUM") as pps:
            with nc.named_scope("warm"):
                for i in range(12):
                    pw = aps.tile([128, 512], F32, tag="sp", name="pw")
                    nc.tensor.matmul(pw[:], wo_sb[:, 0:128], cos_sb[:, 0:512],
                                     start=True, stop=True)
                    if i == 11:
                        wsink = opool.tile([128, 512], F32, tag="wsink", name="wsink")
                        nc.vector.tensor_copy(wsink[:], pw[:])
                        nc.sync.dma_start(out=wscr_d[:], in_=wsink[:])
            for b in range(B):
                ctx_sb = ctx0_sb if b == 0 else ctx1_sb
                with nc.named_scope(f"attn{b}"):
                    for qc in range(2):  # 1024-wide q chunks within this batch
                        q0 = b * S + qc * 1024
                        csl = slice(qc * 1024, (qc + 1) * 1024)
                        for h in range(2):
                            rb = h * 64
                            ctxh = cps.tile([65, 1024], F32, tag="ctx", name="ctx")
                            pring = {}
                            for kb in range(18):
                                if kb < 16:
                                    k0 = b * S + kb * 128
                                    ksl = slice(k0, k0 + 128)
                                    sp_t = aps.tile([128, 1024], F32, tag="sp", name="sp_t")
                                    for half in range(2):
                                        qsl = slice(q0 + half * 512, q0 + (half + 1) * 512)
                                        hsl = slice(half * 512, (half + 1) * 512)
                                        nc.tensor.matmul(
                                            sp_t[:, hsl],
                                            kT_sb[rb:rb + 64, ksl],
                                            qT_sb[rb:rb + 64, qsl],
                                            start=True, stop=True)
                                    p_t = ppool.tile([128, 1024], F16, tag="p", name="p_t")
                                    nc.scalar.activation(
                                        out=p_t[:], in_=sp_t[:],
                                        func=mybir.ActivationFunctionType.Exp)
                                    pring[kb] = p_t
                                    if dbg is not None and b == 0 and qc == 0 and h == 0 and kb == 0:
                                        sp_sb = opool.tile([128, 1024], F32, tag="dbgs")
                                        nc.vector.tensor_copy(sp_sb[:], sp_t[:])
                                        nc.sync.dma_start(out=dbg["sp"][:], in_=sp_sb[:])
                                        nc.gpsimd.dma_start(out=dbg["p"][:], in_=pring[0][:])
                                if kb >= 2:
                                    kv = kb - 2
                                    p_t = pring.pop(kv)
                                    sl_ = h * 32 + b * 16 + kv
                                    st, sp = kv == 0, kv == 15
                                    for half in range(2):
                                        hsl = slice(half * 512, (half + 1) * 512)
                                        nc.tensor.matmul(ctxh[:, hsl], v_all[:, sl_, 0:65],
                                                         p_t[:, hsl], start=st, stop=sp)
                                    if (b, qc, h) != (0, 0, 0) and kv < 12 and projq:
                                        emit_proj_unit(pps, tag="pp")
                            if dbg is not None and b == 0 and qc == 0 and h == 0:
                                cx_sb = opool.tile([65, 1024], F32, tag="dbgc")
                                nc.vector.tensor_copy(cx_sb[:], ctxh[:])
                                nc.sync.dma_start(out=dbg["ctxps"][:], in_=cx_sb[:])
                            # copy ctx psum to SBUF (frees psum), then normalize
                            cuh = lpool.tile([65, 1024], F32, tag=f"cu{h}", name="cuh")
                            nc.vector.tensor_copy(cuh[:], ctxh[:])
                            idx = (b * 2 + qc) * 2 + h
                            lcol = lpool.tile([128, 8], F32, tag="lcol")
                            l0 = cuh[64:65, :]
                            nc.sync.dma_start(
                                out=lcol[:],
                                in_=bass.AP(tensor=l0.tensor, offset=l0.offset,
                                            ap=[list(l0.ap[0]), [8, 128], [1, 8]]))
                            nc.vector.reciprocal(lcol[:], lcol[:])
                            r0 = rscr_d[idx, :]
                            nc.sync.dma_start(
                                out=bass.AP(tensor=r0.tensor, offset=r0.offset,
                                            ap=[[8, 128], [1, 8]]),
                                in_=lcol[:])
                            bct = bpool.tile([64, 1024], F32)
                            nc.sync.dma_start(
                                out=bct[:],
                                in_=bass.AP(tensor=r0.tensor, offset=r0.offset,
                                            ap=[[0, 64], [1, 1024]]))
                            nc.vector.tensor_mul(
                                ctx_sb[rb:rb + 64, csl], cuh[0:64, :], bct[:])
                        for qb in range(qc * 8, (qc + 1) * 8):
                            for oc in range(2):
                                projq.append((b, qb, oc))

        with tc.tile_pool(name="tps", bufs=8, space="PSUM") as tps:
            with nc.named_scope("projtail"):
                i = 0
                while projq:
                    emit_proj_unit(
                        tps,
                        copy_eng=nc.vector if (i // 2) % 2 == 0 else nc.scalar,
                        dma_eng=(nc.sync, nc.scalar)[i % 2],
                        tag="tp")
                    i += 1

        if True:
            if dbg is not None:
                nc.gpsimd.dma_start(out=dbg["qT"][:], in_=qT_sb[:])
                nc.gpsimd.dma_start(out=dbg["kT"][:], in_=kT_sb[:])
                nc.gpsimd.dma_start(out=dbg["v0"][:], in_=v_all[:, 0, :])
                nc.gpsimd.dma_start(out=dbg["ctx0"][:], in_=ctx0_sb[:])
                nc.sync.dma_start(out=dbg["lscr"][:], in_=lscr_d[:])


def _swz(w):
    # [1024, 128] -> [128, 1024]: SBUF layout [p, kc*128+d] = w[kc*128+p, d]
    return np.ascontiguousarray(
        w.reshape(8, 128, 128).transpose(1, 0, 2).reshape(128, 1024))


def _prep_inputs(x, Wq, Wk, Wv, Wo):
    x2 = np.asarray(x, dtype=np.float32).reshape(T, HID)
    xT16 = np.ascontiguousarray(x2.T).astype(np.float16)

    half = HD // 2
    inv_freq = (1.0 / (ROPE_BASE ** (np.arange(half, dtype=np.float64) * 2.0 / HD)))
    ang = np.arange(S, dtype=np.float64)[None, :] * inv_freq[:, None]  # [32, S]
    cosf = np.tile(np.cos(ang), (4, 1)).astype(np.float16)
    sgn = np.repeat([-1.0, 1.0, -1.0, 1.0], 32)[:, None]
    sins = (np.tile(np.sin(ang), (4, 1)) * sgn).astype(np.float16)

    scale = np.float32(1.0 / np.sqrt(HD))
    in_maps = []
    for c in range(NCORES):
        rows = slice(c * DPC, (c + 1) * DPC)
        in_maps.append({
            "xT16": xT16,
            "wq": _swz((Wq[rows, :] * scale).T.astype(np.float16)),
            "wk": _swz(Wk[rows, :].T.astype(np.float16)),
            "wv": _swz(Wv[rows, :].T.astype(np.float16)),
            "wo": np.ascontiguousarray(Wo[:, rows].T).astype(np.float16),
            "cosf": cosf,
            "sins": sins,
        })
    return in_maps


def _run(in_maps, trace=False):
    if "nc" not in _CACHE:
        _CACHE["nc"] = _build_program()
    nc = _CACHE["nc"]
    res = run_bass_kernel_spmd(nc, in_maps, core_ids=list(range(NCORES)),
                               trace=trace)
    acc = res.results[0]["out"].astype(np.float32).copy()
    for c in range(1, NCORES):
        acc += res.results[c]["out"]
    return acc.reshape(B, S, HID), res


def kernel(x, Wq, Wk, Wv, Wo):
    in_maps = _prep_inputs(np.asarray(x), np.asarray(Wq), np.asarray(Wk),
                           np.asarray(Wv), np.asarray(Wo))
    out, _ = _run(in_maps, trace=False)
    return out


def run_profiled(x, Wq, Wk, Wv, Wo):
    in_maps = _prep_inputs(np.asarray(x), np.asarray(Wq), np.asarray(Wk),
                           np.asarray(Wv), np.asarray(Wo))
    return _run(in_maps, trace=True)


# revision 30
# speedup vs baseline: 1.0071x; 1.0071x over previous
"""Multi-head attention with RoPE (B=2, S=2048, H=16 heads, D=64) on 8 TRN2
NeuronCores, tensor-parallel over heads (2 heads/core).

Per core c (heads 2c, 2c+1), all matmul paths in fp16 (fp32 PSUM accum):
  qT/kT = W_slice @ x^T and v (natural layout) from one shared fp16 x^T input
  RoPE on qT/kT: out = t*cos + swap32(t)*sin_signed (swap via SBUF->SBUF DMA)
  scores^T[k, q] = kT.T @ qT per head, exp via ScalarE -> P^T (fp16)
  ctxT[d, q] (+ row of column-sums l) = [v | 1].T @ P^T
  normalize ctxT by 1/l (reciprocal in [128,8] layout, broadcast via DRAM),
  project with Wo slice -> partial output [4096, 1024]; host sums 8 partials.
"""
import numpy as np
import ml_dtypes

import concourse.bass as bass
import concourse.mybir as mybir
import concourse.tile as tile
from concourse import bacc
from concourse.bass_utils import run_bass_kernel_spmd

F32 = mybir.dt.float32
F16 = mybir.dt.float16

B, S, HID = 2, 2048, 1024
NH, HD = 16, 64
T = B * S                  # 4096 tokens
NCORES = 8
HPC = NH // NCORES         # 2 heads per core
DPC = HPC * HD             # 128 context dims per core
ROPE_BASE = 10000.0

_CACHE = {}
DEBUG_TAPS = False


def _build_program():
    nc = bacc.Bacc("TRN2", target_bir_lowering=False, debug=False)

    xT_d = nc.dram_tensor("xT16", [HID, T], F16, kind="ExternalInput")
    wq_d = nc.dram_tensor("wq", [128, HID], F16, kind="ExternalInput")
    wk_d = nc.dram_tensor("wk", [128, HID], F16, kind="ExternalInput")
    wv_d = nc.dram_tensor("wv", [128, HID], F16, kind="ExternalInput")
    wo_d = nc.dram_tensor("wo", [DPC, HID], F16, kind="ExternalInput")
    cos_d = nc.dram_tensor("cosf", [128, S], F16, kind="ExternalInput")
    sin_d = nc.dram_tensor("sins", [128, S], F16, kind="ExternalInput")
    out_d = nc.dram_tensor("out", [T, HID], F32, kind="ExternalOutput")
    lscr_d = nc.dram_tensor("lscr", [8, 1024], F32)   # l rows bounce
    wscr_d = nc.dram_tensor("wscr", [128, 512], F32)  # warmup sink
    rscr_d = nc.dram_tensor("rscr", [8, 1024], F32)   # 1/l rows bounce

    dbg = None
    if DEBUG_TAPS:
        dbg = {
            "qT": nc.dram_tensor("dbg_qT", [128, T], F32, kind="ExternalOutput"),
            "kT": nc.dram_tensor("dbg_kT", [128, T], F32, kind="ExternalOutput"),
            "v0": nc.dram_tensor("dbg_v0", [128, 65], F32, kind="ExternalOutput"),
            "ctx0": nc.dram_tensor("dbg_ctx0", [128, S], F32, kind="ExternalOutput"),
            "lscr": nc.dram_tensor("dbg_lscr", [8, 1024], F32, kind="ExternalOutput"),
            "sp": nc.dram_tensor("dbg_sp", [128, 1024], F32, kind="ExternalOutput"),
            "p": nc.dram_tensor("dbg_p", [128, 1024], F32, kind="ExternalOutput"),
            "ctxps": nc.dram_tensor("dbg_ctxps", [65, 1024], F32, kind="ExternalOutput"),
        }

    with tile.TileContext(nc) as tc:
        _emit(nc, tc, xT_d, wq_d, wk_d, wv_d, wo_d, cos_d, sin_d, out_d,
              lscr_d, rscr_d, wscr_d, dbg=dbg)
    nc.compile()
    return nc


def _emit(nc, tc, xT_d, wq_d, wk_d, wv_d, wo_d, cos_d, sin_d, out_d,
          lscr_d, rscr_d, wscr_d, dbg=None):
    import contextlib
    ctx = contextlib.ExitStack()
    with ctx:
        singles = ctx.enter_context(tc.tile_pool(name="singles", bufs=1))
        xpool = ctx.enter_context(tc.tile_pool(name="xpool", bufs=10))
        ppool = ctx.enter_context(tc.tile_pool(name="ppool", bufs=6))
        rotp = ctx.enter_context(tc.tile_pool(name="rotp", bufs=2))
        lpool = ctx.enter_context(tc.tile_pool(name="lpool", bufs=4))
        bpool = ctx.enter_context(tc.tile_pool(name="bpool", bufs=4))
        opool = ctx.enter_context(tc.tile_pool(name="opool", bufs=12))

        # ---- persistent SBUF ----
        wq_sb = singles.tile([128, 8, DPC], F16)
        wk_sb = singles.tile([128, 8, DPC], F16)
        wv_sb = singles.tile([128, 8, DPC], F16)
        wo_sb = singles.tile([128, HID], F16)
        cos_sb = singles.tile([128, S], F16)
        sin_sb = singles.tile([128, S], F16)
        qT_sb = singles.tile([128, T], F16)
        kT_sb = singles.tile([128, T], F16)
        v_all = singles.tile([128, 64, 65], F16)  # v natural, slot = h*32+b*16+kb
        ctx0_sb = singles.tile([128, S], F16)     # normalized ctx^T for b=0
        ctx1_sb = singles.tile([128, S], F16)

        nc.sync.dma_start(out=wq_sb[:].rearrange("p a b -> p (a b)"), in_=wq_d[:])
        nc.scalar.dma_start(out=wk_sb[:].rearrange("p a b -> p (a b)"), in_=wk_d[:])
        nc.gpsimd.dma_start(out=wv_sb[:].rearrange("p a b -> p (a b)"), in_=wv_d[:])
        nc.gpsimd.dma_start(out=wo_sb[:], in_=wo_d[:])
        nc.gpsimd.dma_start(out=cos_sb[:], in_=cos_d[:])
        nc.gpsimd.dma_start(out=sin_sb[:], in_=sin_d[:])
        nc.vector.memset(v_all[:, :, 64:65], 1.0)
        tblw = singles.tile([1, 8], F32)
        nc.vector.memset(tblw[:], 0.0)
        nc.scalar.activation(out=tblw[:], in_=tblw[:],
                             func=mybir.ActivationFunctionType.Exp)

        # ---- phase 1: q/k (T-layout) + v (natural) from shared x tiles,
        # with RoPE fused per batch-half to keep the PE dense ----
        def rope_one(t_sb, b, part=None):
            # part=None: all; part=k (0..3): quarter k of the DVE mul/add work
            bsl = slice(b * S, (b + 1) * S)
            if part in (None, 0):
                rot = rotp.tile([128, S], F16, tag=f"rot{b}", name="rot")
                rotp_cache[b] = rot
                nc.sync.dma_start(out=rot[0:32, :], in_=t_sb[32:64, bsl])
                nc.sync.dma_start(out=rot[32:64, :], in_=t_sb[0:32, bsl])
                nc.sync.dma_start(out=rot[64:96, :], in_=t_sb[96:128, bsl])
                nc.sync.dma_start(out=rot[96:128, :], in_=t_sb[64:96, bsl])
                nc.gpsimd.tensor_mul(rot[:], rot[:], sin_sb[:])
            rot = rotp_cache[b]
            if part is None:
                cols = [slice(0, S)]
            else:
                half = part % 2
                cols = [slice(half * (S // 2), (half + 1) * (S // 2))]
            for cs in cols:
                tsl2 = slice(b * S + cs.start, b * S + cs.stop)
                nc.vector.tensor_mul(t_sb[:, tsl2], t_sb[:, tsl2], cos_sb[:, cs])
                nc.vector.tensor_add(t_sb[:, tsl2], t_sb[:, tsl2], rot[:, cs])

        rotp_cache = {}

        def rope_half(b):
            rope_one(qT_sb, b)
            rope_one(kT_sb, b)

        with tc.tile_pool(name="qkps", bufs=2, space="PSUM") as qkps, \
             tc.tile_pool(name="vps", bufs=1, space="PSUM") as vps:
            with nc.named_scope("qkv"):
                for tcn in range(8):  # token chunks of 512
                    tsl = slice(tcn * 512, (tcn + 1) * 512)
                    psq = qkps.tile([128, 512], F32)
                    psk = qkps.tile([128, 512], F32)
                    pvs = [vps.tile([128, DPC], F32, tag=f"pv{i}", name=f"pv{i}")
                           for i in range(4)]
                    for kc in range(8):
                        xt = xpool.tile([128, 512], F16)
                        dma_eng = nc.sync if kc % 2 == 0 else nc.scalar
                        dma_eng.dma_start(
                            out=xt[:],
                            in_=xT_d[kc * 128:(kc + 1) * 128, tsl])
                        st, sp = kc == 0, kc == 7
                        nc.tensor.matmul(psq[:], wq_sb[:, kc, :], xt[:], start=st, stop=sp)
                        nc.tensor.matmul(psk[:], wk_sb[:, kc, :], xt[:], start=st, stop=sp)
                        for sub in range(4):
                            nc.tensor.matmul(
                                pvs[sub][:],
                                xt[:, sub * 128:(sub + 1) * 128],
                                wv_sb[:, kc, :],
                                start=st, stop=sp)
                    nc.vector.tensor_copy(qT_sb[:, tsl], psq[:])
                    nc.vector.tensor_copy(kT_sb[:, tsl], psk[:])
                    for sub in range(4):
                        blk = tcn * 4 + sub
                        dst0 = v_all[:, blk, 0:64]
                        dst = bass.AP(tensor=dst0.tensor, offset=dst0.offset,
                                      ap=[list(dst0.ap[0]), [32 * 65, 2], [1, 64]])
                        nc.vector.tensor_copy(dst, pvs[sub][:, 0:128])
                    if tcn == 3:
                        rope_one(qT_sb, 0, part=0)
                    if tcn == 4:
                        rope_one(qT_sb, 0, part=1)
                    if tcn == 5:
                        rope_one(kT_sb, 0, part=0)
                    if tcn == 6:
                        rope_one(kT_sb, 0, part=1)
            with nc.named_scope("rope1"):
                rope_half(1)

        # ---- phases 3+4: attention + output projection, per batch ----
        projq = []

        def emit_proj_unit(pool, copy_eng=None, dma_eng=None, tag="sp"):
            bb, qb, oc = projq.pop(0)
            src = ctx0_sb if bb == 0 else ctx1_sb
            qsl = slice(qb * 128, (qb + 1) * 128)
            osl = slice(oc * 512, (oc + 1) * 512)
            ops = pool.tile([128, 512], F32, tag=tag, name="ops")
            nc.tensor.matmul(ops[:], src[:, qsl], wo_sb[:, osl],
                             start=True, stop=True)
            ot = opool.tile([128, 512], F32, tag="ot", name="ot")
            if copy_eng is nc.scalar:
                nc.scalar.activation(out=ot[:], in_=ops[:],
                                     func=mybir.ActivationFunctionType.Copy)
            else:
                nc.vector.tensor_copy(ot[:], ops[:])
            (dma_eng or nc.sync).dma_start(
                out=out_d[bb * S + qb * 128:bb * S + (qb + 1) * 128, osl],
                in_=ot[:])

        with tc.tile_pool(name="aps", bufs=2, space="PSUM") as aps, \
             tc.tile_pool(name="cps", bufs=1, space="PSUM") as cps, \
             tc.tile_pool(name="pps", bufs=2, space="PSUM") as pps:
            with nc.named_scope("warm"):
                for i in range(12):
                    pw = aps.tile([128, 512], F32, tag="sp", name="pw")
                    nc.tensor.matmul(pw[:], wo_sb[:, 0:128], cos_sb[:, 0:512],
                                     start=True, stop=True)
                    if i == 11:
                        wsink = opool.tile([128, 512], F32, tag="wsink", name="wsink")
                        nc.vector.tensor_copy(wsink[:], pw[:])
                        nc.sync.dma_start(out=wscr_d[:], in_=wsink[:])
            for b in range(B):
                ctx_sb = ctx0_sb if b == 0 else ctx1_sb
                with nc.named_scope(f"attn{b}"):
                    for qc in range(2):  # 1024-wide q chunks within this batch
                        q0 = b * S + qc * 1024
                        csl = slice(qc * 1024, (qc + 1) * 1024)
                        for h in range(2):
                            rb = h * 64
                            ctxh = cps.tile([65, 1024], F32, tag="ctx", name="ctx")
                            pring = {}
                            for kb in range(18):
                                if kb < 16:
                                    k0 = b * S + kb * 128
                                    ksl = slice(k0, k0 + 128)
                                    sp_t = aps.tile([128, 1024], F32, tag="sp", name="sp_t")
                                    for half in range(2):
                                        qsl = slice(q0 + half * 512, q0 + (half + 1) * 512)
                                        hsl = slice(half * 512, (half + 1) * 512)
                                        nc.tensor.matmul(
                                            sp_t[:, hsl],
                                            kT_sb[rb:rb + 64, ksl],
                                            qT_sb[rb:rb + 64, qsl],
                                            start=True, stop=True)
                                    p_t = ppool.tile([128, 1024], F16, tag="p", name="p_t")
                                    nc.scalar.activation(
                                        out=p_t[:], in_=sp_t[:],
                                        func=mybir.ActivationFunctionType.Exp)
                                    pring[kb] = p_t
                                    if dbg is not None and b == 0 and qc == 0 and h == 0 and kb == 0:
                                        sp_sb = opool.tile([128, 1024], F32, tag="dbgs")
                                        nc.vector.tensor_copy(sp_sb[:], sp_t[:])
                                        nc.sync.dma_start(out=dbg["sp"][:], in_=sp_sb[:])
                                        nc.gpsimd.dma_start(out=dbg["p"][:], in_=pring[0][:])
                                if kb >= 2:
                                    kv = kb - 2
                                    p_t = pring.pop(kv)
                                    sl_ = h * 32 + b * 16 + kv
                                    st, sp = kv == 0, kv == 15
                                    for half in range(2):
                                        hsl = slice(half * 512, (half + 1) * 512)
                                        nc.tensor.matmul(ctxh[:, hsl], v_all[:, sl_, 0:65],
                                                         p_t[:, hsl], start=st, stop=sp)
                                    if (b, qc, h) != (0, 0, 0) and kv < 12 and projq:
                                        emit_proj_unit(pps, tag="pp")
                            if dbg is not None and b == 0 and qc == 0 and h == 0:
                                cx_sb = opool.tile([65, 1024], F32, tag="dbgc")
                                nc.vector.tensor_copy(cx_sb[:], ctxh[:])
                                nc.sync.dma_start(out=dbg["ctxps"][:], in_=cx_sb[:])
                            # copy ctx psum to SBUF (frees psum), then normalize
                            cuh = lpool.tile([65, 1024], F32, tag=f"cu{h}", name="cuh")
                            nc.vector.tensor_copy(cuh[:], ctxh[:])
                            idx = (b * 2 + qc) * 2 + h
                            lcol = lpool.tile([128, 8], F32, tag="lcol")
                            l0 = cuh[64:65, :]
                            nc.sync.dma_start(
                                out=lcol[:],
                                in_=bass.AP(tensor=l0.tensor, offset=l0.offset,
                                            ap=[list(l0.ap[0]), [8, 128], [1, 8]]))
                            nc.vector.reciprocal(lcol[:], lcol[:])
                            r0 = rscr_d[idx, :]
                            nc.sync.dma_start(
                                out=bass.AP(tensor=r0.tensor, offset=r0.offset,
                                            ap=[[8, 128], [1, 8]]),
                                in_=lcol[:])
                            bct = bpool.tile([64, 1024], F32)
                            nc.sync.dma_start(
                                out=bct[:],
                                in_=bass.AP(tensor=r0.tensor, offset=r0.offset,
                                            ap=[[0, 64], [1, 1024]]))
                            nc.vector.tensor_mul(
                                ctx_sb[rb:rb + 64, csl], cuh[0:64, :], bct[:])
                        for qb in range(qc * 8, (qc + 1) * 8):
                            for oc in range(2):
                                projq.append((b, qb, oc))

        with tc.tile_pool(name="tps", bufs=8, space="PSUM") as tps:
            with nc.named_scope("projtail"):
                i = 0
                while projq:
                    emit_proj_unit(
                        tps,
                        copy_eng=nc.vector if (i // 2) % 2 == 0 else nc.scalar,
                        dma_eng=(nc.sync, nc.scalar)[i % 2],
                        tag="tp")
                    i += 1

        if True:
            if dbg is not None:
                nc.gpsimd.dma_start(out=dbg["qT"][:], in_=qT_sb[:])
                nc.gpsimd.dma_start(out=dbg["kT"][:], in_=kT_sb[:])
                nc.gpsimd.dma_start(out=dbg["v0"][:], in_=v_all[:, 0, :])
                nc.gpsimd.dma_start(out=dbg["ctx0"][:], in_=ctx0_sb[:])
                nc.sync.dma_start(out=dbg["lscr"][:], in_=lscr_d[:])


def _swz(w):
    # [1024, 128] -> [128, 1024]: SBUF layout [p, kc*128+d] = w[kc*128+p, d]
    return np.ascontiguousarray(
        w.reshape(8, 128, 128).transpose(1, 0, 2).reshape(128, 1024))


def _prep_inputs(x, Wq, Wk, Wv, Wo):
    x2 = np.asarray(x, dtype=np.float32).reshape(T, HID)
    xT16 = np.ascontiguousarray(x2.T).astype(np.float16)

    half = HD // 2
    inv_freq = (1.0 / (ROPE_BASE ** (np.arange(half, dtype=np.float64) * 2.0 / HD)))
    ang = np.arange(S, dtype=np.float64)[None, :] * inv_freq[:, None]  # [32, S]
    cosf = np.tile(np.cos(ang), (4, 1)).astype(np.float16)
    sgn = np.repeat([-1.0, 1.0, -1.0, 1.0], 32)[:, None]
    sins = (np.tile(np.sin(ang), (4, 1)) * sgn).astype(np.float16)

    scale = np.float32(1.0 / np.sqrt(HD))
    in_maps = []
    for c in range(NCORES):
        rows = slice(c * DPC, (c + 1) * DPC)
        in_maps.append({
            "xT16": xT16,
            "wq": _swz((Wq[rows, :] * scale).T.astype(np.float16)),
            "wk": _swz(Wk[rows, :].T.astype(np.float16)),
            "wv": _swz(Wv[rows, :].T.astype(np.float16)),
            "wo": np.ascontiguousarray(Wo[:, rows].T).astype(np.float16),
            "cosf": cosf,
            "sins": sins,
        })
    return in_maps


def _run(in_maps, trace=False):
    if "nc" not in _CACHE:
        _CACHE["nc"] = _build_program()
    nc = _CACHE["nc"]
    res = run_bass_kernel_spmd(nc, in_maps, core_ids=list(range(NCORES)),
                               trace=trace)
    acc = res.results[0]["out"].astype(np.float32).copy()
    for c in range(1, NCORES):
        acc += res.results[c]["out"]
    return acc.reshape(B, S, HID), res


def kernel(x, Wq, Wk, Wv, Wo):
    in_maps = _prep_inputs(np.asarray(x), np.asarray(Wq), np.asarray(Wk),
                           np.asarray(Wv), np.asarray(Wo))
    out, _ = _run(in_maps, trace=False)
    return out


def run_profiled(x, Wq, Wk, Wv, Wo):
    in_maps = _prep_inputs(np.asarray(x), np.asarray(Wq), np.asarray(Wk),
                           np.asarray(Wv), np.asarray(Wo))
    return _run(in_maps, trace=True)


# revision 32
# speedup vs baseline: 1.0089x; 1.0017x over previous
"""Multi-head attention with RoPE (B=2, S=2048, H=16 heads, D=64) on 8 TRN2
NeuronCores, tensor-parallel over heads (2 heads/core).

Per core c (heads 2c, 2c+1), all matmul paths in fp16 (fp32 PSUM accum):
  qT/kT = W_slice @ x^T and v (natural layout) from one shared fp16 x^T input
  RoPE on qT/kT: out = t*cos + swap32(t)*sin_signed (swap via SBUF->SBUF DMA)
  scores^T[k, q] = kT.T @ qT per head, exp via ScalarE -> P^T (fp16)
  ctxT[d, q] (+ row of column-sums l) = [v | 1].T @ P^T
  normalize ctxT by 1/l (reciprocal in [128,8] layout, broadcast via DRAM),
  project with Wo slice -> partial output [4096, 1024]; host sums 8 partials.
"""
import numpy as np
import ml_dtypes

import concourse.bass as bass
import concourse.mybir as mybir
import concourse.tile as tile
from concourse import bacc
from concourse.bass_utils import run_bass_kernel_spmd

F32 = mybir.dt.float32
F16 = mybir.dt.float16

B, S, HID = 2, 2048, 1024
NH, HD = 16, 64
T = B * S                  # 4096 tokens
NCORES = 8
HPC = NH // NCORES         # 2 heads per core
DPC = HPC * HD             # 128 context dims per core
ROPE_BASE = 10000.0

_CACHE = {}
DEBUG_TAPS = False


def _build_program():
    nc = bacc.Bacc("TRN2", target_bir_lowering=False, debug=False)

    xT_d = nc.dram_tensor("xT16", [HID, T], F16, kind="ExternalInput")
    wq_d = nc.dram_tensor("wq", [128, HID], F16, kind="ExternalInput")
    wk_d = nc.dram_tensor("wk", [128, HID], F16, kind="ExternalInput")
    wv_d = nc.dram_tensor("wv", [128, HID], F16, kind="ExternalInput")
    wo_d = nc.dram_tensor("wo", [DPC, HID], F16, kind="ExternalInput")
    cos_d = nc.dram_tensor("cosf", [128, S], F16, kind="ExternalInput")
    sin_d = nc.dram_tensor("sins", [128, S], F16, kind="ExternalInput")
    out_d = nc.dram_tensor("out", [T, HID], F32, kind="ExternalOutput")
    lscr_d = nc.dram_tensor("lscr", [8, 1024], F32)   # l rows bounce
    wscr_d = nc.dram_tensor("wscr", [128, 512], F32)  # warmup sink
    rscr_d = nc.dram_tensor("rscr", [8, 1024], F32)   # 1/l rows bounce

    dbg = None
    if DEBUG_TAPS:
        dbg = {
            "qT": nc.dram_tensor("dbg_qT", [128, T], F32, kind="ExternalOutput"),
            "kT": nc.dram_tensor("dbg_kT", [128, T], F32, kind="ExternalOutput"),
            "v0": nc.dram_tensor("dbg_v0", [128, 65], F32, kind="ExternalOutput"),
            "ctx0": nc.dram_tensor("dbg_ctx0", [128, S], F32, kind="ExternalOutput"),
            "lscr": nc.dram_tensor("dbg_lscr", [8, 1024], F32, kind="ExternalOutput"),
            "sp": nc.dram_tensor("dbg_sp", [128, 1024], F32, kind="ExternalOutput"),
            "p": nc.dram_tensor("dbg_p", [128, 1024], F32, kind="ExternalOutput"),
            "ctxps": nc.dram_tensor("dbg_ctxps", [65, 1024], F32, kind="ExternalOutput"),
        }

    with tile.TileContext(nc) as tc:
        _emit(nc, tc, xT_d, wq_d, wk_d, wv_d, wo_d, cos_d, sin_d, out_d,
              lscr_d, rscr_d, wscr_d, dbg=dbg)
    nc.compile()
    return nc


def _emit(nc, tc, xT_d, wq_d, wk_d, wv_d, wo_d, cos_d, sin_d, out_d,
          lscr_d, rscr_d, wscr_d, dbg=None):
    import contextlib
    ctx = contextlib.ExitStack()
    with ctx:
        singles = ctx.enter_context(tc.tile_pool(name="singles", bufs=1))
        xpool = ctx.enter_context(tc.tile_pool(name="xpool", bufs=10))
        ppool = ctx.enter_context(tc.tile_pool(name="ppool", bufs=6))
        rotp = ctx.enter_context(tc.tile_pool(name="rotp", bufs=2))
        lpool = ctx.enter_context(tc.tile_pool(name="lpool", bufs=4))
        bpool = ctx.enter_context(tc.tile_pool(name="bpool", bufs=4))
        opool = ctx.enter_context(tc.tile_pool(name="opool", bufs=12))

        # ---- persistent SBUF ----
        wq_sb = singles.tile([128, 8, DPC], F16)
        wk_sb = singles.tile([128, 8, DPC], F16)
        wv_sb = singles.tile([128, 8, DPC], F16)
        wo_sb = singles.tile([128, HID], F16)
        cos_sb = singles.tile([128, S], F16)
        sin_sb = singles.tile([128, S], F16)
        qT_sb = singles.tile([128, T], F16)
        kT_sb = singles.tile([128, T], F16)
        v_all = singles.tile([128, 64, 65], F16)  # v natural, slot = h*32+b*16+kb
        ctx0_sb = singles.tile([128, S], F16)     # normalized ctx^T for b=0
        ctx1_sb = singles.tile([128, S], F16)

        nc.sync.dma_start(out=wq_sb[:].rearrange("p a b -> p (a b)"), in_=wq_d[:])
        nc.scalar.dma_start(out=wk_sb[:].rearrange("p a b -> p (a b)"), in_=wk_d[:])
        nc.gpsimd.dma_start(out=wv_sb[:].rearrange("p a b -> p (a b)"), in_=wv_d[:])
        nc.gpsimd.dma_start(out=wo_sb[:], in_=wo_d[:])
        nc.gpsimd.dma_start(out=cos_sb[:], in_=cos_d[:])
        nc.gpsimd.dma_start(out=sin_sb[:], in_=sin_d[:])
        nc.vector.memset(v_all[:, :, 64:65], 1.0)
        tblw = singles.tile([1, 8], F32)
        nc.vector.memset(tblw[:], 0.0)
        nc.scalar.activation(out=tblw[:], in_=tblw[:],
                             func=mybir.ActivationFunctionType.Exp)

        # ---- phase 1: q/k (T-layout) + v (natural) from shared x tiles,
        # with RoPE fused per batch-half to keep the PE dense ----
        def rope_one(t_sb, b, part=None):
            # part=None: all; part=k (0..3): quarter k of the DVE mul/add work
            bsl = slice(b * S, (b + 1) * S)
            if part in (None, 0):
                rot = rotp.tile([128, S], F16, tag=f"rot{b}", name="rot")
                rotp_cache[b] = rot
                nc.sync.dma_start(out=rot[0:32, :], in_=t_sb[32:64, bsl])
                nc.sync.dma_start(out=rot[32:64, :], in_=t_sb[0:32, bsl])
                nc.sync.dma_start(out=rot[64:96, :], in_=t_sb[96:128, bsl])
                nc.sync.dma_start(out=rot[96:128, :], in_=t_sb[64:96, bsl])
                nc.gpsimd.tensor_mul(rot[:], rot[:], sin_sb[:])
            rot = rotp_cache[b]
            if part is None:
                cols = [slice(0, S)]
            else:
                half = part % 2
                cols = [slice(half * (S // 2), (half + 1) * (S // 2))]
            for cs in cols:
                tsl2 = slice(b * S + cs.start, b * S + cs.stop)
                nc.vector.tensor_mul(t_sb[:, tsl2], t_sb[:, tsl2], cos_sb[:, cs])
                nc.vector.tensor_add(t_sb[:, tsl2], t_sb[:, tsl2], rot[:, cs])

        rotp_cache = {}

        def rope_half(b):
            rope_one(qT_sb, b)
            rope_one(kT_sb, b)

        with tc.tile_pool(name="qkps", bufs=2, space="PSUM") as qkps, \
             tc.tile_pool(name="vps", bufs=1, space="PSUM") as vps:
            with nc.named_scope("qkv"):
                for tcn in range(8):  # token chunks of 512
                    tsl = slice(tcn * 512, (tcn + 1) * 512)
                    psq = qkps.tile([128, 512], F32)
                    psk = qkps.tile([128, 512], F32)
                    pvs = [vps.tile([128, DPC], F32, tag=f"pv{i}", name=f"pv{i}")
                           for i in range(4)]
                    for kc in range(8):
                        xt = xpool.tile([128, 512], F16)
                        dma_eng = nc.sync if kc % 2 == 0 else nc.scalar
                        dma_eng.dma_start(
                            out=xt[:],
                            in_=xT_d[kc * 128:(kc + 1) * 128, tsl])
                        st, sp = kc == 0, kc == 7
                        nc.tensor.matmul(psq[:], wq_sb[:, kc, :], xt[:], start=st, stop=sp)
                        nc.tensor.matmul(psk[:], wk_sb[:, kc, :], xt[:], start=st, stop=sp)
                        for sub in range(4):
                            nc.tensor.matmul(
                                pvs[sub][:],
                                xt[:, sub * 128:(sub + 1) * 128],
                                wv_sb[:, kc, :],
                                start=st, stop=sp)
                    nc.vector.tensor_copy(qT_sb[:, tsl], psq[:])
                    nc.vector.tensor_copy(kT_sb[:, tsl], psk[:])
                    for sub in range(4):
                        blk = tcn * 4 + sub
                        dst0 = v_all[:, blk, 0:64]
                        dst = bass.AP(tensor=dst0.tensor, offset=dst0.offset,
                                      ap=[list(dst0.ap[0]), [32 * 65, 2], [1, 64]])
                        nc.vector.tensor_copy(dst, pvs[sub][:, 0:128])
                    if tcn == 3:
                        rope_one(qT_sb, 0, part=0)
                    if tcn == 4:
                        rope_one(qT_sb, 0, part=1)
                    if tcn == 5:
                        rope_one(kT_sb, 0, part=0)
                    if tcn == 6:
                        rope_one(kT_sb, 0, part=1)
            with nc.named_scope("rope1"):
                rope_half(1)

        # ---- phases 3+4: attention + output projection, per batch ----
        projq = []

        def emit_proj_unit(pool, copy_eng=None, dma_eng=None, tag="sp"):
            bb, qb, oc = projq.pop(0)
            src = ctx0_sb if bb == 0 else ctx1_sb
            qsl = slice(qb * 128, (qb + 1) * 128)
            osl = slice(oc * 512, (oc + 1) * 512)
            ops = pool.tile([128, 512], F32, tag=tag, name="ops")
            nc.tensor.matmul(ops[:], src[:, qsl], wo_sb[:, osl],
                             start=True, stop=True)
            ot = opool.tile([128, 512], F32, tag="ot", name="ot")
            if copy_eng is nc.scalar:
                nc.scalar.activation(out=ot[:], in_=ops[:],
                                     func=mybir.ActivationFunctionType.Copy)
            else:
                nc.vector.tensor_copy(ot[:], ops[:])
            (dma_eng or nc.sync).dma_start(
                out=out_d[bb * S + qb * 128:bb * S + (qb + 1) * 128, osl],
                in_=ot[:])

        with tc.tile_pool(name="aps", bufs=2, space="PSUM") as aps, \
             tc.tile_pool(name="cps", bufs=1, space="PSUM") as cps:
            with nc.named_scope("warm"):
                for i in range(12):
                    pw = aps.tile([128, 512], F32, tag="sp", name="pw")
                    nc.tensor.matmul(pw[:], wo_sb[:, 0:128], cos_sb[:, 0:512],
                                     start=True, stop=True)
                    if i == 11:
                        wsink = opool.tile([128, 512], F32, tag="wsink", name="wsink")
                        nc.vector.tensor_copy(wsink[:], pw[:])
                        nc.sync.dma_start(out=wscr_d[:], in_=wsink[:])
            for b in range(B):
                ctx_sb = ctx0_sb if b == 0 else ctx1_sb
                with nc.named_scope(f"attn{b}"):
                    for qc in range(2):  # 1024-wide q chunks within this batch
                        q0 = b * S + qc * 1024
                        csl = slice(qc * 1024, (qc + 1) * 1024)
                        for h in range(2):
                            rb = h * 64
                            ctxh = cps.tile([65, 1024], F32, tag=f"ctx{(qc * 2 + h) % 2}", name="ctx")
                            pring = {}
                            for kb in range(18):
                                if kb < 16:
                                    k0 = b * S + kb * 128
                                    ksl = slice(k0, k0 + 128)
                                    sp_t = aps.tile([128, 1024], F32, tag="sp", name="sp_t")
                                    for half in range(2):
                                        qsl = slice(q0 + half * 512, q0 + (half + 1) * 512)
                                        hsl = slice(half * 512, (half + 1) * 512)
                                        nc.tensor.matmul(
                                            sp_t[:, hsl],
                                            kT_sb[rb:rb + 64, ksl],
                                            qT_sb[rb:rb + 64, qsl],
                                            start=True, stop=True)
                                    p_t = ppool.tile([128, 1024], F16, tag="p", name="p_t")
                                    nc.scalar.activation(
                                        out=p_t[:], in_=sp_t[:],
                                        func=mybir.ActivationFunctionType.Exp)
                                    pring[kb] = p_t
                                    if dbg is not None and b == 0 and qc == 0 and h == 0 and kb == 0:
                                        sp_sb = opool.tile([128, 1024], F32, tag="dbgs")
                                        nc.vector.tensor_copy(sp_sb[:], sp_t[:])
                                        nc.sync.dma_start(out=dbg["sp"][:], in_=sp_sb[:])
                                        nc.gpsimd.dma_start(out=dbg["p"][:], in_=pring[0][:])
                                if kb >= 2:
                                    kv = kb - 2
                                    p_t = pring.pop(kv)
                                    sl_ = h * 32 + b * 16 + kv
                                    st, sp = kv == 0, kv == 15
                                    for half in range(2):
                                        hsl = slice(half * 512, (half + 1) * 512)
                                        nc.tensor.matmul(ctxh[:, hsl], v_all[:, sl_, 0:65],
                                                         p_t[:, hsl], start=st, stop=sp)
                                    if (b, qc, h) != (0, 0, 0) and kv < 8 and projq:
                                        emit_proj_unit(
                                            cps, tag=f"ctx{(qc * 2 + h + 1) % 2}")
                            if dbg is not None and b == 0 and qc == 0 and h == 0:
                                cx_sb = opool.tile([65, 1024], F32, tag="dbgc")
                                nc.vector.tensor_copy(cx_sb[:], ctxh[:])
                                nc.sync.dma_start(out=dbg["ctxps"][:], in_=cx_sb[:])
                            # copy ctx psum to SBUF (frees psum), then normalize
                            cuh = lpool.tile([65, 1024], F32, tag=f"cu{h}", name="cuh")
                            nc.vector.tensor_copy(cuh[:], ctxh[:])
                            idx = (b * 2 + qc) * 2 + h
                            lcol = lpool.tile([128, 8], F32, tag="lcol")
                            l0 = cuh[64:65, :]
                            nc.sync.dma_start(
                                out=lcol[:],
                                in_=bass.AP(tensor=l0.tensor, offset=l0.offset,
                                            ap=[list(l0.ap[0]), [8, 128], [1, 8]]))
                            nc.vector.reciprocal(lcol[:], lcol[:])
                            r0 = rscr_d[idx, :]
                            nc.sync.dma_start(
                                out=bass.AP(tensor=r0.tensor, offset=r0.offset,
                                            ap=[[8, 128], [1, 8]]),
                                in_=lcol[:])
                            bct = bpool.tile([64, 1024], F32)
                            nc.sync.dma_start(
                                out=bct[:],
                                in_=bass.AP(tensor=r0.tensor, offset=r0.offset,
                                            ap=[[0, 64], [1, 1024]]))
                            nc.vector.tensor_mul(
                                ctx_sb[rb:rb + 64, csl], cuh[0:64, :], bct[:])
                        for qb in range(qc * 8, (qc + 1) * 8):
                            for oc in range(2):
                                projq.append((b, qb, oc))

        with tc.tile_pool(name="tps", bufs=8, space="PSUM") as tps:
            with nc.named_scope("projtail"):
                i = 0
                while projq:
                    emit_proj_unit(
                        tps,
                        copy_eng=nc.vector if (i // 2) % 2 == 0 else nc.scalar,
                        dma_eng=(nc.sync, nc.scalar)[i % 2],
                        tag="tp")
                    i += 1

        if True:
            if dbg is not None:
                nc.gpsimd.dma_start(out=dbg["qT"][:], in_=qT_sb[:])
                nc.gpsimd.dma_start(out=dbg["kT"][:], in_=kT_sb[:])
                nc.gpsimd.dma_start(out=dbg["v0"][:], in_=v_all[:, 0, :])
                nc.gpsimd.dma_start(out=dbg["ctx0"][:], in_=ctx0_sb[:])
                nc.sync.dma_start(out=dbg["lscr"][:], in_=lscr_d[:])


def _swz(w):
    # [1024, 128] -> [128, 1024]: SBUF layout [p, kc*128+d] = w[kc*128+p, d]
    return np.ascontiguousarray(
        w.reshape(8, 128, 128).transpose(1, 0, 2).reshape(128, 1024))


def _prep_inputs(x, Wq, Wk, Wv, Wo):
    x2 = np.asarray(x, dtype=np.float32).reshape(T, HID)
    xT16 = np.ascontiguousarray(x2.T).astype(np.float16)

    half = HD // 2
    inv_freq = (1.0 / (ROPE_BASE ** (np.arange(half, dtype=np.float64) * 2.0 / HD)))
    ang = np.arange(S, dtype=np.float64)[None, :] * inv_freq[:, None]  # [32, S]
    cosf = np.tile(np.cos(ang), (4, 1)).astype(np.float16)
    sgn = np.repeat([-1.0, 1.0, -1.0, 1.0], 32)[:, None]
    sins = (np.tile(np.sin(ang), (4, 1)) * sgn).astype(np.float16)

    scale = np.float32(1.0 / np.sqrt(HD))
    in_maps = []
    for c in range(NCORES):
        rows = slice(c * DPC, (c + 1) * DPC)
        in_maps.append({
            "xT16": xT16,
            "wq": _swz((Wq[rows, :] * scale).T.astype(np.float16)),
            "wk": _swz(Wk[rows, :].T.astype(np.float16)),
            "wv": _swz(Wv[rows, :].T.astype(np.float16)),
            "wo": np.ascontiguousarray(Wo[:, rows].T).astype(np.float16),
            "cosf": cosf,
            "sins": sins,
        })
    return in_maps


def _run(in_maps, trace=False):
    if "nc" not in _CACHE:
        _CACHE["nc"] = _build_program()
    nc = _CACHE["nc"]
    res = run_bass_kernel_spmd(nc, in_maps, core_ids=list(range(NCORES)),
                               trace=trace)
    acc = res.results[0]["out"].astype(np.float32).copy()
    for c in range(1, NCORES):
        acc += res.results[c]["out"]
    return acc.reshape(B, S, HID), res


def kernel(x, Wq, Wk, Wv, Wo):
    in_maps = _prep_inputs(np.asarray(x), np.asarray(Wq), np.asarray(Wk),
                           np.asarray(Wv), np.asarray(Wo))
    out, _ = _run(in_maps, trace=False)
    return out


def run_profiled(x, Wq, Wk, Wv, Wo):
    in_maps = _prep_inputs(np.asarray(x), np.asarray(Wq), np.asarray(Wk),
                           np.asarray(Wv), np.asarray(Wo))
    return _run(in_maps, trace=True)


# revision 33
# speedup vs baseline: 1.0109x; 1.0020x over previous
"""Multi-head attention with RoPE (B=2, S=2048, H=16 heads, D=64) on 8 TRN2
NeuronCores, tensor-parallel over heads (2 heads/core).

Per core c (heads 2c, 2c+1), all matmul paths in fp16 (fp32 PSUM accum):
  qT/kT = W_slice @ x^T and v (natural layout) from one shared fp16 x^T input
  RoPE on qT/kT: out = t*cos + swap32(t)*sin_signed (swap via SBUF->SBUF DMA)
  scores^T[k, q] = kT.T @ qT per head, exp via ScalarE -> P^T (fp16)
  ctxT[d, q] (+ row of column-sums l) = [v | 1].T @ P^T
  normalize ctxT by 1/l (reciprocal in [128,8] layout, broadcast via DRAM),
  project with Wo slice -> partial output [4096, 1024]; host sums 8 partials.
"""
import numpy as np
import ml_dtypes

import concourse.bass as bass
import concourse.mybir as mybir
import concourse.tile as tile
from concourse import bacc
from concourse.bass_utils import run_bass_kernel_spmd

F32 = mybir.dt.float32
F16 = mybir.dt.float16

B, S, HID = 2, 2048, 1024
NH, HD = 16, 64
T = B * S                  # 4096 tokens
NCORES = 8
HPC = NH // NCORES         # 2 heads per core
DPC = HPC * HD             # 128 context dims per core
ROPE_BASE = 10000.0

_CACHE = {}
DEBUG_TAPS = False


def _build_program():
    nc = bacc.Bacc("TRN2", target_bir_lowering=False, debug=False)

    xT_d = nc.dram_tensor("xT16", [HID, T], F16, kind="ExternalInput")
    wq_d = nc.dram_tensor("wq", [128, HID], F16, kind="ExternalInput")
    wk_d = nc.dram_tensor("wk", [128, HID], F16, kind="ExternalInput")
    wv_d = nc.dram_tensor("wv", [128, HID], F16, kind="ExternalInput")
    wo_d = nc.dram_tensor("wo", [DPC, HID], F16, kind="ExternalInput")
    cos_d = nc.dram_tensor("cosf", [128, S], F16, kind="ExternalInput")
    sin_d = nc.dram_tensor("sins", [128, S], F16, kind="ExternalInput")
    out_d = nc.dram_tensor("out", [T, HID], F32, kind="ExternalOutput")
    lscr_d = nc.dram_tensor("lscr", [8, 1024], F32)   # l rows bounce
    wscr_d = nc.dram_tensor("wscr", [128, 512], F32)  # warmup sink
    rscr_d = nc.dram_tensor("rscr", [8, 1024], F32)   # 1/l rows bounce

    dbg = None
    if DEBUG_TAPS:
        dbg = {
            "qT": nc.dram_tensor("dbg_qT", [128, T], F32, kind="ExternalOutput"),
            "kT": nc.dram_tensor("dbg_kT", [128, T], F32, kind="ExternalOutput"),
            "v0": nc.dram_tensor("dbg_v0", [128, 65], F32, kind="ExternalOutput"),
            "ctx0": nc.dram_tensor("dbg_ctx0", [128, S], F32, kind="ExternalOutput"),
            "lscr": nc.dram_tensor("dbg_lscr", [8, 1024], F32, kind="ExternalOutput"),
            "sp": nc.dram_tensor("dbg_sp", [128, 1024], F32, kind="ExternalOutput"),
            "p": nc.dram_tensor("dbg_p", [128, 1024], F32, kind="ExternalOutput"),
            "ctxps": nc.dram_tensor("dbg_ctxps", [65, 1024], F32, kind="ExternalOutput"),
        }

    with tile.TileContext(nc) as tc:
        _emit(nc, tc, xT_d, wq_d, wk_d, wv_d, wo_d, cos_d, sin_d, out_d,
              lscr_d, rscr_d, wscr_d, dbg=dbg)
    nc.compile()
    return nc


def _emit(nc, tc, xT_d, wq_d, wk_d, wv_d, wo_d, cos_d, sin_d, out_d,
          lscr_d, rscr_d, wscr_d, dbg=None):
    import contextlib
    ctx = contextlib.ExitStack()
    with ctx:
        singles = ctx.enter_context(tc.tile_pool(name="singles", bufs=1))
        xpool = ctx.enter_context(tc.tile_pool(name="xpool", bufs=10))
        ppool = ctx.enter_context(tc.tile_pool(name="ppool", bufs=6))
        rotp = ctx.enter_context(tc.tile_pool(name="rotp", bufs=2))
        lpool = ctx.enter_context(tc.tile_pool(name="lpool", bufs=4))
        bpool = ctx.enter_context(tc.tile_pool(name="bpool", bufs=4))
        opool = ctx.enter_context(tc.tile_pool(name="opool", bufs=12))

        # ---- persistent SBUF ----
        wq_sb = singles.tile([128, 8, DPC], F16)
        wk_sb = singles.tile([128, 8, DPC], F16)
        wv_sb = singles.tile([128, 8, DPC], F16)
        wo_sb = singles.tile([128, HID], F16)
        cos_sb = singles.tile([128, S], F16)
        sin_sb = singles.tile([128, S], F16)
        qT_sb = singles.tile([128, T], F16)
        kT_sb = singles.tile([128, T], F16)
        v_all = singles.tile([128, 64, 65], F16)  # v natural, slot = h*32+b*16+kb
        ctx0_sb = singles.tile([128, S], F16)     # normalized ctx^T for b=0
        ctx1_sb = singles.tile([128, S], F16)

        nc.sync.dma_start(out=wq_sb[:].rearrange("p a b -> p (a b)"), in_=wq_d[:])
        nc.scalar.dma_start(out=wk_sb[:].rearrange("p a b -> p (a b)"), in_=wk_d[:])
        nc.gpsimd.dma_start(out=wv_sb[:].rearrange("p a b -> p (a b)"), in_=wv_d[:])
        nc.gpsimd.dma_start(out=wo_sb[:], in_=wo_d[:])
        nc.gpsimd.dma_start(out=cos_sb[:], in_=cos_d[:])
        nc.gpsimd.dma_start(out=sin_sb[:], in_=sin_d[:])
        nc.vector.memset(v_all[:, :, 64:65], 1.0)
        tblw = singles.tile([1, 8], F32)
        nc.vector.memset(tblw[:], 0.0)
        nc.scalar.activation(out=tblw[:], in_=tblw[:],
                             func=mybir.ActivationFunctionType.Exp)

        # ---- phase 1: q/k (T-layout) + v (natural) from shared x tiles,
        # with RoPE fused per batch-half to keep the PE dense ----
        def rope_one(t_sb, b, part=None):
            # part=None: all; part=k (0..3): quarter k of the DVE mul/add work
            bsl = slice(b * S, (b + 1) * S)
            if part in (None, 0):
                rot = rotp.tile([128, S], F16, tag=f"rot{b}", name="rot")
                rotp_cache[b] = rot
                nc.sync.dma_start(out=rot[0:32, :], in_=t_sb[32:64, bsl])
                nc.sync.dma_start(out=rot[32:64, :], in_=t_sb[0:32, bsl])
                nc.sync.dma_start(out=rot[64:96, :], in_=t_sb[96:128, bsl])
                nc.sync.dma_start(out=rot[96:128, :], in_=t_sb[64:96, bsl])
                nc.gpsimd.tensor_mul(rot[:], rot[:], sin_sb[:])
            rot = rotp_cache[b]
            if part is None:
                cols = [slice(0, S)]
            else:
                half = part % 2
                cols = [slice(half * (S // 2), (half + 1) * (S // 2))]
            for cs in cols:
                tsl2 = slice(b * S + cs.start, b * S + cs.stop)
                nc.vector.tensor_mul(t_sb[:, tsl2], t_sb[:, tsl2], cos_sb[:, cs])
                nc.vector.tensor_add(t_sb[:, tsl2], t_sb[:, tsl2], rot[:, cs])

        rotp_cache = {}

        def rope_half(b):
            rope_one(qT_sb, b)
            rope_one(kT_sb, b)

        with tc.tile_pool(name="qkps", bufs=2, space="PSUM") as qkps, \
             tc.tile_pool(name="vps", bufs=1, space="PSUM") as vps:
            with nc.named_scope("qkv"):
                for tcn in range(8):  # token chunks of 512
                    tsl = slice(tcn * 512, (tcn + 1) * 512)
                    psq = qkps.tile([128, 512], F32)
                    psk = qkps.tile([128, 512], F32)
                    pvs = [vps.tile([128, DPC], F32, tag=f"pv{i}", name=f"pv{i}")
                           for i in range(4)]
                    for kc in range(8):
                        xt = xpool.tile([128, 512], F16)
                        dma_eng = nc.sync if kc % 2 == 0 else nc.scalar
                        dma_eng.dma_start(
                            out=xt[:],
                            in_=xT_d[kc * 128:(kc + 1) * 128, tsl])
                        st, sp = kc == 0, kc == 7
                        nc.tensor.matmul(psq[:], wq_sb[:, kc, :], xt[:], start=st, stop=sp)
                        nc.tensor.matmul(psk[:], wk_sb[:, kc, :], xt[:], start=st, stop=sp)
                        for sub in range(4):
                            nc.tensor.matmul(
                                pvs[sub][:],
                                xt[:, sub * 128:(sub + 1) * 128],
                                wv_sb[:, kc, :],
                                start=st, stop=sp)
                    nc.scalar.activation(out=qT_sb[:, tsl], in_=psq[:],
                                         func=mybir.ActivationFunctionType.Copy)
                    nc.scalar.activation(out=kT_sb[:, tsl], in_=psk[:],
                                         func=mybir.ActivationFunctionType.Copy)
                    for sub in range(4):
                        blk = tcn * 4 + sub
                        dst0 = v_all[:, blk, 0:64]
                        dst = bass.AP(tensor=dst0.tensor, offset=dst0.offset,
                                      ap=[list(dst0.ap[0]), [32 * 65, 2], [1, 64]])
                        nc.vector.tensor_copy(dst, pvs[sub][:, 0:128])
                    if tcn == 3:
                        rope_one(qT_sb, 0, part=0)
                    if tcn == 4:
                        rope_one(qT_sb, 0, part=1)
                    if tcn == 5:
                        rope_one(kT_sb, 0, part=0)
                    if tcn == 6:
                        rope_one(kT_sb, 0, part=1)
            with nc.named_scope("rope1"):
                rope_half(1)

        # ---- phases 3+4: attention + output projection, per batch ----
        projq = []

        def emit_proj_unit(pool, copy_eng=None, dma_eng=None, tag="sp"):
            bb, qb, oc = projq.pop(0)
            src = ctx0_sb if bb == 0 else ctx1_sb
            qsl = slice(qb * 128, (qb + 1) * 128)
            osl = slice(oc * 512, (oc + 1) * 512)
            ops = pool.tile([128, 512], F32, tag=tag, name="ops")
            nc.tensor.matmul(ops[:], src[:, qsl], wo_sb[:, osl],
                             start=True, stop=True)
            ot = opool.tile([128, 512], F32, tag="ot", name="ot")
            if copy_eng is nc.scalar:
                nc.scalar.activation(out=ot[:], in_=ops[:],
                                     func=mybir.ActivationFunctionType.Copy)
            else:
                nc.vector.tensor_copy(ot[:], ops[:])
            (dma_eng or nc.sync).dma_start(
                out=out_d[bb * S + qb * 128:bb * S + (qb + 1) * 128, osl],
                in_=ot[:])

        with tc.tile_pool(name="aps", bufs=2, space="PSUM") as aps, \
             tc.tile_pool(name="cps", bufs=1, space="PSUM") as cps:
            with nc.named_scope("warm"):
                for i in range(12):
                    pw = aps.tile([128, 512], F32, tag="sp", name="pw")
                    nc.tensor.matmul(pw[:], wo_sb[:, 0:128], cos_sb[:, 0:512],
                                     start=True, stop=True)
                    if i == 11:
                        wsink = opool.tile([128, 512], F32, tag="wsink", name="wsink")
                        nc.vector.tensor_copy(wsink[:], pw[:])
                        nc.sync.dma_start(out=wscr_d[:], in_=wsink[:])
            for b in range(B):
                ctx_sb = ctx0_sb if b == 0 else ctx1_sb
                with nc.named_scope(f"attn{b}"):
                    for qc in range(2):  # 1024-wide q chunks within this batch
                        q0 = b * S + qc * 1024
                        csl = slice(qc * 1024, (qc + 1) * 1024)
                        for h in range(2):
                            rb = h * 64
                            ctxh = cps.tile([65, 1024], F32, tag=f"ctx{(qc * 2 + h) % 2}", name="ctx")
                            pring = {}
                            for kb in range(18):
                                if kb < 16:
                                    k0 = b * S + kb * 128
                                    ksl = slice(k0, k0 + 128)
                                    sp_t = aps.tile([128, 1024], F32, tag="sp", name="sp_t")
                                    for half in range(2):
                                        qsl = slice(q0 + half * 512, q0 + (half + 1) * 512)
                                        hsl = slice(half * 512, (half + 1) * 512)
                                        nc.tensor.matmul(
                                            sp_t[:, hsl],
                                            kT_sb[rb:rb + 64, ksl],
                                            qT_sb[rb:rb + 64, qsl],
                                            start=True, stop=True)
                                    p_t = ppool.tile([128, 1024], F16, tag="p", name="p_t")
                                    nc.scalar.activation(
                                        out=p_t[:], in_=sp_t[:],
                                        func=mybir.ActivationFunctionType.Exp)
                                    pring[kb] = p_t
                                    if dbg is not None and b == 0 and qc == 0 and h == 0 and kb == 0:
                                        sp_sb = opool.tile([128, 1024], F32, tag="dbgs")
                                        nc.vector.tensor_copy(sp_sb[:], sp_t[:])
                                        nc.sync.dma_start(out=dbg["sp"][:], in_=sp_sb[:])
                                        nc.gpsimd.dma_start(out=dbg["p"][:], in_=pring[0][:])
                                if kb >= 2:
                                    kv = kb - 2
                                    p_t = pring.pop(kv)
                                    sl_ = h * 32 + b * 16 + kv
                                    st, sp = kv == 0, kv == 15
                                    for half in range(2):
                                        hsl = slice(half * 512, (half + 1) * 512)
                                        nc.tensor.matmul(ctxh[:, hsl], v_all[:, sl_, 0:65],
                                                         p_t[:, hsl], start=st, stop=sp)
                                    if (b, qc, h) != (0, 0, 0) and kv < 8 and projq:
                                        emit_proj_unit(
                                            cps, tag=f"ctx{(qc * 2 + h + 1) % 2}")
                            if dbg is not None and b == 0 and qc == 0 and h == 0:
                                cx_sb = opool.tile([65, 1024], F32, tag="dbgc")
                                nc.vector.tensor_copy(cx_sb[:], ctxh[:])
                                nc.sync.dma_start(out=dbg["ctxps"][:], in_=cx_sb[:])
                            # copy ctx psum to SBUF (frees psum), then normalize
                            cuh = lpool.tile([65, 1024], F32, tag=f"cu{h}", name="cuh")
                            nc.vector.tensor_copy(cuh[:], ctxh[:])
                            idx = (b * 2 + qc) * 2 + h
                            lcol = lpool.tile([128, 8], F32, tag="lcol")
                            l0 = cuh[64:65, :]
                            nc.sync.dma_start(
                                out=lcol[:],
                                in_=bass.AP(tensor=l0.tensor, offset=l0.offset,
                                            ap=[list(l0.ap[0]), [8, 128], [1, 8]]))
                            nc.vector.reciprocal(lcol[:], lcol[:])
                            r0 = rscr_d[idx, :]
                            nc.sync.dma_start(
                                out=bass.AP(tensor=r0.tensor, offset=r0.offset,
                                            ap=[[8, 128], [1, 8]]),
                                in_=lcol[:])
                            bct = bpool.tile([64, 1024], F32)
                            nc.sync.dma_start(
                                out=bct[:],
                                in_=bass.AP(tensor=r0.tensor, offset=r0.offset,
                                            ap=[[0, 64], [1, 1024]]))
                            nc.vector.tensor_mul(
                                ctx_sb[rb:rb + 64, csl], cuh[0:64, :], bct[:])
                        for qb in range(qc * 8, (qc + 1) * 8):
                            for oc in range(2):
                                projq.append((b, qb, oc))

        with tc.tile_pool(name="tps", bufs=8, space="PSUM") as tps:
            with nc.named_scope("projtail"):
                i = 0
                while projq:
                    emit_proj_unit(
                        tps,
                        copy_eng=nc.vector if (i // 2) % 2 == 0 else nc.scalar,
                        dma_eng=(nc.sync, nc.scalar)[i % 2],
                        tag="tp")
                    i += 1

        if True:
            if dbg is not None:
                nc.gpsimd.dma_start(out=dbg["qT"][:], in_=qT_sb[:])
                nc.gpsimd.dma_start(out=dbg["kT"][:], in_=kT_sb[:])
                nc.gpsimd.dma_start(out=dbg["v0"][:], in_=v_all[:, 0, :])
                nc.gpsimd.dma_start(out=dbg["ctx0"][:], in_=ctx0_sb[:])
                nc.sync.dma_start(out=dbg["lscr"][:], in_=lscr_d[:])


def _swz(w):
    # [1024, 128] -> [128, 1024]: SBUF layout [p, kc*128+d] = w[kc*128+p, d]
    return np.ascontiguousarray(
        w.reshape(8, 128, 128).transpose(1, 0, 2).reshape(128, 1024))


def _prep_inputs(x, Wq, Wk, Wv, Wo):
    x2 = np.asarray(x, dtype=np.float32).reshape(T, HID)
    xT16 = np.ascontiguousarray(x2.T).astype(np.float16)

    half = HD // 2
    inv_freq = (1.0 / (ROPE_BASE ** (np.arange(half, dtype=np.float64) * 2.0 / HD)))
    ang = np.arange(S, dtype=np.float64)[None, :] * inv_freq[:, None]  # [32, S]
    cosf = np.tile(np.cos(ang), (4, 1)).astype(np.float16)
    sgn = np.repeat([-1.0, 1.0, -1.0, 1.0], 32)[:, None]
    sins = (np.tile(np.sin(ang), (4, 1)) * sgn).astype(np.float16)

    scale = np.float32(1.0 / np.sqrt(HD))
    in_maps = []
    for c in range(NCORES):
        rows = slice(c * DPC, (c + 1) * DPC)
        in_maps.append({
            "xT16": xT16,
            "wq": _swz((Wq[rows, :] * scale).T.astype(np.float16)),
            "wk": _swz(Wk[rows, :].T.astype(np.float16)),
            "wv": _swz(Wv[rows, :].T.astype(np.float16)),
            "wo": np.ascontiguousarray(Wo[:, rows].T).astype(np.float16),
            "cosf": cosf,
            "sins": sins,
        })
    return in_maps


def _run(in_maps, trace=False):
    if "nc" not in _CACHE:
        _CACHE["nc"] = _build_program()
    nc = _CACHE["nc"]
    res = run_bass_kernel_spmd(nc, in_maps, core_ids=list(range(NCORES)),
                               trace=trace)
    acc = res.results[0]["out"].astype(np.float32).copy()
    for c in range(1, NCORES):
        acc += res.results[c]["out"]
    return acc.reshape(B, S, HID), res


def kernel(x, Wq, Wk, Wv, Wo):
    in_maps = _prep_inputs(np.asarray(x), np.asarray(Wq), np.asarray(Wk),
                           np.asarray(Wv), np.asarray(Wo))
    out, _ = _run(in_maps, trace=False)
    return out


def run_profiled(x, Wq, Wk, Wv, Wo):
    in_maps = _prep_inputs(np.asarray(x), np.asarray(Wq), np.asarray(Wk),
                           np.asarray(Wv), np.asarray(Wo))
    return _run(in_maps, trace=True)
